# revision 2
# baseline (speedup 1.0000x reference)
"""Trainium2 Bass kernel for 3x3 valid conv (C_in=8, C_out=8, H=W=2048).

v2 strategy (spatial H-sharding across 8 cores, fp8e3 activations,
column-parity deinterleave):
  - Host splits x rows into 8 slabs of 256 output rows (+2 halo input rows),
    quantizes to fp8 e3m4 (exact-RNE; measured rel err 0.0145 < 2e-2 gate with
    fp16 weights), and packs each slab into SBUF layout
        xp[(ci, r, u), b, n] = slab[ci, 6*b + r, 2*n + u]
    for 43 row-blocks b (last block starts at row 250), r = 0..7 input rows,
    u = column parity. K = 8*8*2 = 128 contraction partitions.
  - Two lhsT weight matrices (one per rhs shift s in {0,1}) of shape
    [K=128, M=96], M = (co, j, v) with j = 0..5 output rows, v = out parity:
        lhsT[s][(ci,r,u), (co,j,v)] = W[co, ci, r-j, 2s+u-v]
    when 0 <= r-j <= 2 and 0 <= 2s+u-v <= 2. Each (kh, kw) tap appears in
    exactly one s, so 2 accumulating passes/block replace 3 (and N is halved
    by the parity packing): PE work drops from 3*2046 to 2*1023 cols/block,
    with 43 blocks of 6 rows instead of 19 of 14.
  - Device per core: per group of 4 blocks one DMA loads [128, 4*1024] fp8;
    per block, per output-pair tile (512/511 wide), 2 accumulating matmuls
    (shift s=0,1) produce [96, <=512] fp32 PSUM; PSUM tiles are copied
    (DVE/ACT alternating, fp32->fp16) into [96, 4*1023] and stored with one
    DMA into op[(co,j,v), b, n]. Host scatters op back to (C, 2046, 2046).
"""

import numpy as np
import ml_dtypes

import concourse.bass as bass
import concourse.mybir as mybir
import concourse.tile as tile
from concourse import bacc
from concourse.bass_utils import run_bass_kernel_spmd

# ---- problem geometry (hardcoded) ----
C = 8
H = 2048
W = 2048
KH = KW = 3
H_OUT = H - KH + 1   # 2046
W_OUT = W - KW + 1   # 2046
N_CORES = 8

ROWS_PER_CORE = 256          # output rows computed per core (core 7: 254 valid)
IN_ROWS = ROWS_PER_CORE + 2  # 258 input rows per core slab

J = 6                        # output rows per block
R = J + 2                    # 8 input rows per block
U = 2                        # column parity deinterleave
K = C * R * U                # 128 contraction partitions: k = ci*16 + r*2 + u
M = C * J * U                # 96 output partitions:      m = co*12 + j*2 + v
NBLK = 43                    # blocks per core (42*6=252, last block at 250)
BLOCK_STARTS = [J * b for b in range(NBLK - 1)] + [ROWS_PER_CORE - J]
NPAIR = W // U               # 1024 input column pairs per row
OPAIR = W_OUT // U           # 1023 output column pairs per row

COL_TILES = [(0, 512), (512, 511)]

IN_DT = mybir.dt.float8e3    # on-wire activation dtype
IN_NP = ml_dtypes.float8_e3m4
W_DT = mybir.dt.float16      # weight dtype
W_NP = np.float16
OUT_DT = mybir.dt.float16    # on-wire output dtype (host upcasts)
OUT_NP = np.float16

GRP = 4                      # max blocks per DMA group
# group sizes: small head groups (fast pipeline start), small tail groups
# (short drain after last matmul)
GROUP_SIZES = [1, 2, 3] + [4] * 8 + [2, 2, 1]
assert sum(GROUP_SIZES) == NBLK
Y_BUFS = 8
O_BUFS = 4
N_WARM = 14                  # PE-warmup matmuls during the first load wait


def build_nc(repeat: int = 1, mode: str = "full", grp: int = GRP,
             group_sizes=None, y_bufs: int = Y_BUFS, o_bufs: int = O_BUFS,
             load_eng: str = "pool", ps_bufs: int = 8, n_warm: int = N_WARM,
             warm_n: int = 128):
    do_mm = mode in ("full", "nocopy")
    do_copy = mode in ("full",)
    do_dma = mode in ("full", "nocopy", "dma")
    if group_sizes is None:
        group_sizes = GROUP_SIZES
    groups = []
    s = 0
    for gs in group_sizes:
        groups.append(list(range(s, s + gs)))
        s += gs
    assert s == NBLK
    nc = bacc.Bacc(
        "TRN2",
        target_bir_lowering=False,
        debug=False,
        num_devices=N_CORES,
    )
    xp = nc.dram_tensor("xp", [K, NBLK, NPAIR], IN_DT, kind="ExternalInput").ap()
    wts = nc.dram_tensor("wts", [K, U * M], W_DT, kind="ExternalInput").ap()
    op = nc.dram_tensor("op", [M, NBLK, OPAIR], OUT_DT, kind="ExternalOutput").ap()

    with tile.TileContext(nc) as tc:
        with (
            tc.tile_pool(name="wpool", bufs=1) as wpool,
            tc.tile_pool(name="ypool", bufs=y_bufs) as ypool,
            tc.tile_pool(name="opool", bufs=o_bufs) as opool,
            tc.tile_pool(name="pspool", bufs=ps_bufs, space="PSUM") as pspool,
        ):
            # weights (one small DMA) then group-0 load, both on the fast
            # HWDGE (sync) queue so the first matmul's operands are in flight
            # before anything else
            wsb = wpool.tile([K, U * M], W_DT)
            nc.sync.dma_start(wsb[:], wts)
            g0 = len(groups[0])
            y0 = ypool.tile([K, g0 * NPAIR], IN_DT, name="y", tag="y",
                            padded_shape=[K, grp * NPAIR])
            if do_dma:
                nc.sync.dma_start(y0[:], xp[:, 0:g0, :])

            # PE warm-up: the tensor clock ramps 0.65->1.2->2.4 GHz with
            # sustained use; burn the first-load wait on dummy matmuls over a
            # zeroed scratch tile so the real matmuls start at full clock.
            if n_warm and do_mm:
                wscr = wpool.tile([K, max(M, warm_n)], W_DT)
                nc.vector.memset(wscr[:], 0.0)
                psw = pspool.tile([M, 512], mybir.dt.float32, name="psw", tag="ps")
                for _ in range(n_warm):
                    nc.tensor.matmul(psw[:, :warm_n], lhsT=wscr[:, :M],
                                     rhs=wscr[:, :warm_n], start=True, stop=True)

            for rep_i in range(repeat):
                for gi, blocks in enumerate(groups):
                    g = len(blocks)
                    b0 = blocks[0]
                    if rep_i == 0 and gi == 0:
                        yt = y0
                    else:
                        yt = ypool.tile([K, g * NPAIR], IN_DT, name="y", tag="y",
                                        padded_shape=[K, grp * NPAIR])
                        if do_dma:
                            eng = nc.gpsimd if load_eng == "pool" else nc.sync
                            eng.dma_start(yt[:], xp[:, b0:b0 + g, :])
                    ys = [yt]

                    o = opool.tile([M, g * OPAIR], OUT_DT, name="o", tag="o",
                                   padded_shape=[M, grp * OPAIR])
                    for bi in range(g):
                        pss = []
                        for ti, (n0, nn) in enumerate(COL_TILES):
                            ps = pspool.tile([M, 512], mybir.dt.float32,
                                             name=f"ps{ti}", tag="ps")
                            pss.append(ps)
                        if do_mm:
                            y = ys[0]
                            yb = bi
                            # s-outer: both col tiles share the stationary side
                            for s in range(U):
                                for ti, (n0, nn) in enumerate(COL_TILES):
                                    c0 = yb * NPAIR + n0 + s
                                    nc.tensor.matmul(
                                        pss[ti][:, :nn],
                                        lhsT=wsb[:, s * M:(s + 1) * M],
                                        rhs=y[:, c0:c0 + nn],
                                        start=(s == 0),
                                        stop=(s == U - 1),
                                    )
                        if do_copy:
                            for ti, (n0, nn) in enumerate(COL_TILES):
                                dst = o[:, bi * OPAIR + n0:bi * OPAIR + n0 + nn]
                                if (bi + ti) % 2 == 0:
                                    nc.vector.tensor_copy(dst, pss[ti][:, :nn])
                                else:
                                    nc.scalar.copy(dst, pss[ti][:, :nn])
                    if not do_copy and do_dma:
                        # ablation modes: cheap writer so Tile allocates o
                        nc.vector.memset(o[:, :8], 0.0)
                    if do_dma:
                        if do_copy and gi == len(groups) - 1 and g == 1:
                            # split the final store per col tile on two queues
                            # so the kernel tail is one small store, not a
                            # copy->full-store chain
                            nc.sync.dma_start(op[:, b0:b0 + 1, 0:512],
                                              o[:, 0:512])
                            nc.scalar.dma_start(op[:, b0:b0 + 1, 512:OPAIR],
                                                o[:, 512:OPAIR])
                        else:
                            nc.sync.dma_start(op[:, b0:b0 + g, :], o[:])

    nc.compile()
    return nc


def build_weight_lhst(weight: np.ndarray) -> np.ndarray:
    """weight: (C_out, C_in, 3, 3) fp32 -> (K, U*M) fp16 ([:, s*M:(s+1)*M]
    is the lhsT for shift s)."""
    wl = np.zeros((U, K, M), np.float32)
    ci = np.arange(C)
    for s in range(U):
        for co in range(C):
            for j in range(J):
                for v in range(U):
                    for kh in range(KH):
                        for kw in range(KW):
                            u = kw - 2 * s + v
                            if 0 <= u < U:
                                r = j + kh
                                wl[s, ci * (R * U) + r * U + u,
                                   co * (J * U) + j * U + v] = weight[co, :, kh, kw]
    return wl.transpose(1, 0, 2).reshape(K, U * M).astype(W_NP)


def pack_core_input(slab: np.ndarray) -> np.ndarray:
    """slab: (C, IN_ROWS, W) fp8 -> xp (K, NBLK, NPAIR) fp8."""
    s0, s1, s2 = slab.strides
    # xp[ci, r, u, b, n] = slab[ci, 6b + r, 2n + u]; b = 0..41 uniform stride,
    # b = 42 special (start 250, overlap-recompute tail)
    v = np.lib.stride_tricks.as_strided(
        slab, shape=(C, R, U, NBLK - 1, NPAIR),
        strides=(s0, s1, s2, J * s1, U * s2),
    )
    xp = np.empty((C, R, U, NBLK, NPAIR), slab.dtype)
    xp[:, :, :, :NBLK - 1, :] = v
    last = slab[:, BLOCK_STARTS[-1]:BLOCK_STARTS[-1] + R, :]  # (C, R, W)
    xp[:, :, :, NBLK - 1, :] = last.reshape(C, R, NPAIR, U).transpose(0, 1, 3, 2)
    return xp.reshape(K, NBLK, NPAIR)


def unpack_core_output(op: np.ndarray) -> np.ndarray:
    """op: (M, NBLK, OPAIR) -> (C, ROWS_PER_CORE, W_OUT) float32."""
    op = op.reshape(C, J, U, NBLK, OPAIR)
    res = np.empty((C, ROWS_PER_CORE, W_OUT), np.float32)
    # last block: rows 250..255
    lastb = op[:, :, :, NBLK - 1, :].transpose(0, 1, 3, 2).reshape(C, J, W_OUT)
    # main blocks 0..41: rows 0..251
    main = op[:, :, :, :NBLK - 1, :].transpose(0, 3, 1, 4, 2).reshape(
        C, J * (NBLK - 1), W_OUT)
    res[:, :J * (NBLK - 1), :] = main
    res[:, BLOCK_STARTS[-1]:, :] = lastb
    return res


def shard_inputs(x: np.ndarray, weight: np.ndarray):
    xq = np.ascontiguousarray(x).astype(IN_NP)
    wl = build_weight_lhst(weight)
    in_maps = []
    for i in range(N_CORES):
        lo = i * ROWS_PER_CORE
        hi = min(lo + IN_ROWS, H)
        if hi - lo == IN_ROWS:
            slab = xq[:, lo:hi, :]
        else:
            slab = np.zeros((C, IN_ROWS, W), IN_NP)
            slab[:, :hi - lo, :] = xq[:, lo:hi, :]
        in_maps.append({"xp": pack_core_input(slab), "wts": wl})
    return in_maps


def unshard_output(results) -> np.ndarray:
    parts = []
    for i in range(N_CORES):
        rows = ROWS_PER_CORE if i < N_CORES - 1 else H_OUT - (N_CORES - 1) * ROWS_PER_CORE
        parts.append(unpack_core_output(results[i]["op"])[:, :rows, :])
    return np.concatenate(parts, axis=1)


_NC_CACHE = None


def _get_nc():
    global _NC_CACHE
    if _NC_CACHE is None:
        _NC_CACHE = build_nc()
    return _NC_CACHE


def run(inputs: dict, **spmd_kwargs):
    """Run the conv on 8 NeuronCores. Returns (full_output, BassKernelResults)."""
    in_maps = shard_inputs(np.asarray(inputs["x"]), np.asarray(inputs["weight"]))
    nc = _get_nc()
    res = run_bass_kernel_spmd(nc, in_maps, core_ids=list(range(N_CORES)), **spmd_kwargs)
    return unshard_output(res.results).astype(np.float32), res


def kernel(**inputs) -> np.ndarray:
    out, _ = run(inputs)
    return out


# revision 4
# speedup vs baseline: 1.4580x; 1.4580x over previous
"""Trainium2 Bass kernel for 3x3 valid conv (C_in=8, C_out=8, H=W=2048).

v2 strategy (spatial H-sharding across 8 cores, fp8e3 activations,
column-parity deinterleave):
  - Host splits x rows into 8 slabs of 256 output rows (+2 halo input rows),
    quantizes to fp8 e3m4 (exact-RNE; measured rel err 0.0145 < 2e-2 gate with
    fp16 weights), and packs each slab into SBUF layout
        xp[(ci, r, u), b, n] = slab[ci, 6*b + r, 2*n + u]
    for 43 row-blocks b (last block starts at row 250), r = 0..7 input rows,
    u = column parity. K = 8*8*2 = 128 contraction partitions.
  - Two lhsT weight matrices (one per rhs shift s in {0,1}) of shape
    [K=128, M=96], M = (co, j, v) with j = 0..5 output rows, v = out parity:
        lhsT[s][(ci,r,u), (co,j,v)] = W[co, ci, r-j, 2s+u-v]
    when 0 <= r-j <= 2 and 0 <= 2s+u-v <= 2. Each (kh, kw) tap appears in
    exactly one s, so 2 accumulating passes/block replace 3 (and N is halved
    by the parity packing): PE work drops from 3*2046 to 2*1023 cols/block,
    with 43 blocks of 6 rows instead of 19 of 14.
  - Device per core: per group of (up to) 4 blocks one DMA loads
    [128, g*1024] fp8 (group sizes taper at the stream head/tail for fast
    pipeline ramp and short drain; uniform 4s in the repeated body);
    per block, per output-pair tile (512/511 wide), 2 accumulating matmuls
    (shift s=0,1) produce [96, <=512] fp32 PSUM; PSUM tiles are copied
    (DVE/ACT alternating, fp32->fp16) into [96, 4*1023] and stored with one
    DMA into op[(co,j,v), b, n]. Host scatters op back to (C, 2046, 2046).
"""

import numpy as np
import ml_dtypes

import concourse.bass as bass
import concourse.mybir as mybir
import concourse.tile as tile
from concourse import bacc
from concourse.bass_utils import run_bass_kernel_spmd

# ---- problem geometry (hardcoded) ----
C = 8
H = 2048
W = 2048
KH = KW = 3
H_OUT = H - KH + 1   # 2046
W_OUT = W - KW + 1   # 2046
N_CORES = 8

ROWS_PER_CORE = 256          # output rows computed per core (core 7: 254 valid)
IN_ROWS = ROWS_PER_CORE + 2  # 258 input rows per core slab

J = 6                        # output rows per block
R = J + 2                    # 8 input rows per block
U = 2                        # column parity deinterleave
K = C * R * U                # 128 contraction partitions: k = ci*16 + r*2 + u
M = C * J * U                # 96 output partitions:      m = co*12 + j*2 + v
NBLK = 43                    # blocks per core (42*6=252, last block at 250)
BLOCK_STARTS = [J * b for b in range(NBLK - 1)] + [ROWS_PER_CORE - J]
NPAIR = W // U               # 1024 input column pairs per row
OPAIR = W_OUT // U           # 1023 output column pairs per row

COL_TILES = [(0, 512), (512, 511)]

IN_DT = mybir.dt.float8e3    # on-wire activation dtype
IN_NP = ml_dtypes.float8_e3m4
W_DT = mybir.dt.float16      # weight dtype
W_NP = np.float16
OUT_DT = mybir.dt.float16    # on-wire output dtype (host upcasts)
OUT_NP = np.float16

GRP = 4                      # max blocks per DMA group
# group sizes: small head groups (fast pipeline start), small tail groups
# (short drain after last matmul)
GROUP_SIZES = [1, 2, 3] + [4] * 8 + [2, 2, 1]
assert sum(GROUP_SIZES) == NBLK
Y_BUFS = 10
O_BUFS = 6
N_WARM = 14                  # PE-warmup matmuls during the first load wait


def build_nc(repeat: int = 1, mode: str = "full", grp: int = GRP,
             group_sizes=None, y_bufs: int = Y_BUFS, o_bufs: int = O_BUFS,
             load_eng: str = "pool", ps_bufs: int = 8, n_warm: int = N_WARM,
             warm_n: int = 128):
    do_mm = mode in ("full", "nocopy")
    do_copy = mode in ("full",)
    do_dma = mode in ("full", "nocopy", "dma")
    def make_groups(sizes):
        out, s = [], 0
        for gs in sizes:
            out.append(list(range(s, s + gs)))
            s += gs
        assert s == NBLK
        return out

    def sizes_for(rep_i):
        if group_sizes is not None:
            return group_sizes
        first, last = rep_i == 0, rep_i == repeat - 1
        if first and last:
            return GROUP_SIZES              # ramp head + tapered tail
        if first:
            return [1, 2, 3] + [4] * 9 + [1]
        if last:
            return [4] * 10 + [2, 1]
        return [4] * 10 + [3]               # steady state: fewest DMAs
    nc = bacc.Bacc(
        "TRN2",
        target_bir_lowering=False,
        debug=False,
        num_devices=N_CORES,
    )
    xp = nc.dram_tensor("xp", [K, NBLK, NPAIR], IN_DT, kind="ExternalInput").ap()
    wts = nc.dram_tensor("wts", [K, U * M], W_DT, kind="ExternalInput").ap()
    op = nc.dram_tensor("op", [M, NBLK, OPAIR], OUT_DT, kind="ExternalOutput").ap()

    with tile.TileContext(nc) as tc:
        with (
            tc.tile_pool(name="wpool", bufs=1) as wpool,
            tc.tile_pool(name="ypool", bufs=y_bufs) as ypool,
            tc.tile_pool(name="opool", bufs=o_bufs) as opool,
            tc.tile_pool(name="pspool", bufs=ps_bufs, space="PSUM") as pspool,
        ):
            # weights (one small DMA) then group-0 load, both in flight
            # before anything else
            wsb = wpool.tile([K, U * M], W_DT)
            nc.sync.dma_start(wsb[:], wts)
            groups = make_groups(sizes_for(0))
            g0 = len(groups[0])
            y0 = ypool.tile([K, g0 * NPAIR], IN_DT, name="y", tag="y",
                            padded_shape=[K, grp * NPAIR])
            if do_dma:
                # SWDGE (pool) path runs in parallel with the weights' HWDGE
                # pipeline; two sync DMAs would serialize in the HWDGE ring
                nc.gpsimd.dma_start(y0[:], xp[:, 0:g0, :])

            # PE warm-up: the tensor clock ramps 0.65->1.2->2.4 GHz with
            # sustained use; burn the first-load wait on dummy matmuls over a
            # zeroed scratch tile so the real matmuls start at full clock.
            if n_warm and do_mm:
                wscr = wpool.tile([K, max(M, warm_n)], W_DT)
                nc.vector.memset(wscr[:], 0.0)
                psw = pspool.tile([M, 512], mybir.dt.float32, name="psw", tag="ps")
                for _ in range(n_warm):
                    nc.tensor.matmul(psw[:, :warm_n], lhsT=wscr[:, :M],
                                     rhs=wscr[:, :warm_n], start=True, stop=True)

            for rep_i in range(repeat):
                groups = make_groups(sizes_for(rep_i))
                for gi, blocks in enumerate(groups):
                    g = len(blocks)
                    b0 = blocks[0]
                    if rep_i == 0 and gi == 0:
                        yt = y0
                    else:
                        yt = ypool.tile([K, g * NPAIR], IN_DT, name="y", tag="y",
                                        padded_shape=[K, grp * NPAIR])
                        if do_dma:
                            eng = nc.gpsimd if load_eng == "pool" else nc.sync
                            eng.dma_start(yt[:], xp[:, b0:b0 + g, :])
                    ys = [yt]

                    o = opool.tile([M, g * OPAIR], OUT_DT, name="o", tag="o",
                                   padded_shape=[M, grp * OPAIR])
                    for bi in range(g):
                        pss = []
                        for ti, (n0, nn) in enumerate(COL_TILES):
                            ps = pspool.tile([M, 512], mybir.dt.float32,
                                             name=f"ps{ti}", tag="ps")
                            pss.append(ps)
                        if do_mm:
                            y = ys[0]
                            yb = bi
                            # s-outer: both col tiles share the stationary side
                            for s in range(U):
                                for ti, (n0, nn) in enumerate(COL_TILES):
                                    c0 = yb * NPAIR + n0 + s
                                    nc.tensor.matmul(
                                        pss[ti][:, :nn],
                                        lhsT=wsb[:, s * M:(s + 1) * M],
                                        rhs=y[:, c0:c0 + nn],
                                        start=(s == 0),
                                        stop=(s == U - 1),
                                    )
                        if do_copy:
                            for ti, (n0, nn) in enumerate(COL_TILES):
                                dst = o[:, bi * OPAIR + n0:bi * OPAIR + n0 + nn]
                                if (bi + ti) % 2 == 0:
                                    nc.vector.tensor_copy(dst, pss[ti][:, :nn])
                                else:
                                    nc.scalar.copy(dst, pss[ti][:, :nn])
                    if not do_copy and do_dma:
                        # ablation modes: cheap writer so Tile allocates o
                        nc.vector.memset(o[:, :8], 0.0)
                    if do_dma:
                        if do_copy and gi == len(groups) - 1 and g == 1:
                            # split the final store per col tile on two queues
                            # so the kernel tail is one small store, not a
                            # copy->full-store chain
                            nc.sync.dma_start(op[:, b0:b0 + 1, 0:512],
                                              o[:, 0:512])
                            nc.scalar.dma_start(op[:, b0:b0 + 1, 512:OPAIR],
                                                o[:, 512:OPAIR])
                        else:
                            nc.sync.dma_start(op[:, b0:b0 + g, :], o[:])

    nc.compile()
    return nc


def build_weight_lhst(weight: np.ndarray) -> np.ndarray:
    """weight: (C_out, C_in, 3, 3) fp32 -> (K, U*M) fp16 ([:, s*M:(s+1)*M]
    is the lhsT for shift s)."""
    wl = np.zeros((U, K, M), np.float32)
    ci = np.arange(C)
    for s in range(U):
        for co in range(C):
            for j in range(J):
                for v in range(U):
                    for kh in range(KH):
                        for kw in range(KW):
                            u = kw - 2 * s + v
                            if 0 <= u < U:
                                r = j + kh
                                wl[s, ci * (R * U) + r * U + u,
                                   co * (J * U) + j * U + v] = weight[co, :, kh, kw]
    return wl.transpose(1, 0, 2).reshape(K, U * M).astype(W_NP)


def pack_core_input(slab: np.ndarray) -> np.ndarray:
    """slab: (C, IN_ROWS, W) fp8 -> xp (K, NBLK, NPAIR) fp8."""
    s0, s1, s2 = slab.strides
    # xp[ci, r, u, b, n] = slab[ci, 6b + r, 2n + u]; b = 0..41 uniform stride,
    # b = 42 special (start 250, overlap-recompute tail)
    v = np.lib.stride_tricks.as_strided(
        slab, shape=(C, R, U, NBLK - 1, NPAIR),
        strides=(s0, s1, s2, J * s1, U * s2),
    )
    xp = np.empty((C, R, U, NBLK, NPAIR), slab.dtype)
    xp[:, :, :, :NBLK - 1, :] = v
    last = slab[:, BLOCK_STARTS[-1]:BLOCK_STARTS[-1] + R, :]  # (C, R, W)
    xp[:, :, :, NBLK - 1, :] = last.reshape(C, R, NPAIR, U).transpose(0, 1, 3, 2)
    return xp.reshape(K, NBLK, NPAIR)


def unpack_core_output(op: np.ndarray) -> np.ndarray:
    """op: (M, NBLK, OPAIR) -> (C, ROWS_PER_CORE, W_OUT) float32."""
    op = op.reshape(C, J, U, NBLK, OPAIR)
    res = np.empty((C, ROWS_PER_CORE, W_OUT), np.float32)
    # last block: rows 250..255
    lastb = op[:, :, :, NBLK - 1, :].transpose(0, 1, 3, 2).reshape(C, J, W_OUT)
    # main blocks 0..41: rows 0..251
    main = op[:, :, :, :NBLK - 1, :].transpose(0, 3, 1, 4, 2).reshape(
        C, J * (NBLK - 1), W_OUT)
    res[:, :J * (NBLK - 1), :] = main
    res[:, BLOCK_STARTS[-1]:, :] = lastb
    return res


def shard_inputs(x: np.ndarray, weight: np.ndarray):
    xq = np.ascontiguousarray(x).astype(IN_NP)
    wl = build_weight_lhst(weight)
    in_maps = []
    for i in range(N_CORES):
        lo = i * ROWS_PER_CORE
        hi = min(lo + IN_ROWS, H)
        if hi - lo == IN_ROWS:
            slab = xq[:, lo:hi, :]
        else:
            slab = np.zeros((C, IN_ROWS, W), IN_NP)
            slab[:, :hi - lo, :] = xq[:, lo:hi, :]
        in_maps.append({"xp": pack_core_input(slab), "wts": wl})
    return in_maps


def unshard_output(results) -> np.ndarray:
    parts = []
    for i in range(N_CORES):
        rows = ROWS_PER_CORE if i < N_CORES - 1 else H_OUT - (N_CORES - 1) * ROWS_PER_CORE
        parts.append(unpack_core_output(results[i]["op"])[:, :rows, :])
    return np.concatenate(parts, axis=1)


_NC_CACHE = None


def _get_nc():
    global _NC_CACHE
    if _NC_CACHE is None:
        _NC_CACHE = build_nc()
    return _NC_CACHE


def run(inputs: dict, **spmd_kwargs):
    """Run the conv on 8 NeuronCores. Returns (full_output, BassKernelResults)."""
    in_maps = shard_inputs(np.asarray(inputs["x"]), np.asarray(inputs["weight"]))
    nc = _get_nc()
    res = run_bass_kernel_spmd(nc, in_maps, core_ids=list(range(N_CORES)), **spmd_kwargs)
    return unshard_output(res.results).astype(np.float32), res


def kernel(**inputs) -> np.ndarray:
    out, _ = run(inputs)
    return out
